# revision 37
# baseline (speedup 1.0000x reference)
"""Trainium2 Bass kernel for batched self-attention with input projections.

Problem: B=8, N=2048, D=131
    Q = q @ Wq.T + bq;  K = k @ Wk.T + bk;  V = v @ Wv.T + bv
    out = softmax(Q K^T / sqrt(131)) V

One batch element per NeuronCore (8 cores, no communication).

Host prep (layout/algebra only):
  - Tokens augmented with a ones-row: X = [x^T; 1] in [132, 2048] so biases
    fold into the projection matmuls.
  - Scores: Q K^T = Xq (Wq'^T Wk'/sqrt(D)) Xk^T = Xq G Xk^T, G [132,132].
    SVD-truncate G to rank 128 (exact rank 131; error ~2e-5) so the big S
    matmul is a single K=128 contraction:  S = (Xq Aq)(Xk Ak)^T.
  - Value path: W2 [132,132] maps X -> [V | 1] (bias row + denominator
    ones-column).  SVD-truncate W2 = L R^T to rank 128 so the O-matmul
    contracts into a 128-wide latent:  O' = (P Xv L) R^T, with O'[:,131]
    the softmax denominator.

Device schedule (the critical resource is the ACT engine: 32 exps of
[128,1024], ~1.0us each):
  - Input DMA issues are split across the Sync and ACT sequencers in
    need-order (wpack/K/Q first, V last) so the exp stream starts ~11us.
  - ACT's exp table is preloaded via a dummy exp before its DMA issues.
  - exp output and the V-latent are fp8e4; the O-matmul runs DoubleRow
    (two key-blocks packed per matmul, K=256) so the PE keeps pace.
  - Output written bf16 in a partition-major layout (one DMA packet per
    partition row) and depermuted on the host.
"""

import numpy as np
import ml_dtypes

P = 128          # partitions / PE width
N = 2048         # tokens per core
D = 131          # embed dim
DP = 132         # embed dim + ones row
DLO = DP - P     # tail contraction rows (4)
R = 128          # truncated rank (QK interaction and V latent)
EV = 132         # final output cols (131 + denominator)
NB = N // P      # 16 key blocks
NPAIR = NB // 2  # 8 key-block pairs (DoubleRow)
HW = 1024        # query-half width
NH = N // HW     # 2 halves
NIB = HW // P    # 8 i-blocks per half
NCORES = 8

QOFF, KOFF, VOFF = 0, N, 2 * N          # column offsets in packed xall
AQOFF, AKOFF, LOFF = 0, R, 2 * R        # column offsets in packed weights

_BF16 = ml_dtypes.bfloat16

USE_FP8 = True   # fp8e4 E/VL (faster O-matmul, ~2.5x the error)
USE_DR = True   # DoubleRow perf mode for the O-matmul


def build_nc():
    """Build the single-core Bass graph (same NEFF runs SPMD on all 8 cores)."""
    from contextlib import ExitStack

    import concourse.bacc as bacc
    import concourse.mybir as mybir
    import concourse.tile as tile
    from concourse.bass import ts

    bf = mybir.dt.bfloat16
    f32 = mybir.dt.float32
    f8 = mybir.dt.float8e4
    EXP = mybir.ActivationFunctionType.Exp
    COPY = mybir.ActivationFunctionType.Copy
    DR = mybir.MatmulPerfMode.DoubleRow

    nc = bacc.Bacc()
    xall = nc.declare_dram_parameter("xall", [DP, 3 * N], bf, isOutput=False)
    wpack = nc.declare_dram_parameter("wpack", [DP, 3 * R], bf, isOutput=False)
    rmat = nc.declare_dram_parameter("rmat", [R, EV], bf, isOutput=False)
    # out[h, p, ib, e] = token (h*1024 + ib*128 + p), feature e
    out = nc.declare_dram_parameter("out", [NH, P, NIB, D], bf, isOutput=True)

    with tile.TileContext(nc) as tc, ExitStack() as ctx:
        const = ctx.enter_context(tc.tile_pool(name="const", bufs=1))
        xin = ctx.enter_context(tc.tile_pool(name="xin", bufs=1))
        proj = ctx.enter_context(tc.tile_pool(name="proj", bufs=1))
        epool = ctx.enter_context(tc.tile_pool(name="epool", bufs=12))
        ohs = ctx.enter_context(tc.tile_pool(name="ohs", bufs=1))
        outp = ctx.enter_context(tc.tile_pool(name="outp", bufs=1))
        nrm = ctx.enter_context(tc.tile_pool(name="nrm", bufs=4))
        warm = ctx.enter_context(tc.tile_pool(name="warm", bufs=1))
        # PSUM (8 banks): psp 2x[128,512]=2, psst 2x[128,1024]=4,
        # psoh 1x[128,1024]=2.
        psp = ctx.enter_context(tc.tile_pool(name="psp", bufs=2, space="PSUM"))
        psst = ctx.enter_context(tc.tile_pool(name="psst", bufs=2, space="PSUM"))
        psoh = ctx.enter_context(tc.tile_pool(name="psoh", bufs=1, space="PSUM"))

        # ---- SBUF tiles
        wp_hi = const.tile([P, 3 * R], bf)
        wp_lo = const.tile([DLO, 3 * R], bf)
        rmat_s = const.tile([R, EV], bf)
        xall_hi = xin.tile([P, 3 * N], bf)
        xall_lo = xin.tile([DLO, 3 * N], bf)
        edt = f8 if USE_FP8 else bf
        qts = proj.tile([P, N], bf, tag="qts", name="qts")  # QT [latent, tok]
        kts = proj.tile([P, N], bf, tag="kts", name="kts")  # KT [latent, tok]
        vlp = [proj.tile([P, 2, P], edt, tag=f"vl{p}", name=f"vl{p}")
               for p in range(NPAIR)]                       # VL [tok, 2, lat]

        # ---- ACT program: preload the Exp table via a dummy activation,
        # then issue the DMAs that the Sync sequencer would otherwise
        # serialize behind its own.
        zt = warm.tile([P, 1], f32)
        nc.gpsimd.memset(zt, 0)
        dummy = warm.tile([P, 1], f32)
        nc.scalar.activation(dummy, zt, EXP)
        nc.scalar.dma_start(out=wp_hi, in_=wpack[0:P, :])
        nc.scalar.dma_start(out=wp_lo, in_=wpack[P:DP, :])
        nc.scalar.dma_start(out=rmat_s, in_=rmat[:, :])

        # ---- Sync DMA issues in need-order.  Packets drain per-ring in
        # issue order, so this sequence is the arrival schedule.
        def dma_hi(xoff, c0, c1):
            nc.sync.dma_start(
                out=xall_hi[:, xoff + c0:xoff + c1],
                in_=xall[0:P, xoff + c0:xoff + c1],
            )

        def dma_lo(xoff):
            nc.sync.dma_start(
                out=xall_lo[:, xoff:xoff + N],
                in_=xall[P:DP, xoff:xoff + N],
            )

        dma_hi(KOFF, 0, 512)
        dma_lo(KOFF)
        dma_hi(QOFF, 0, 512)
        dma_lo(QOFF)
        dma_hi(QOFF, 512, HW)
        dma_hi(KOFF, 512, HW)
        dma_hi(KOFF, HW, N)
        dma_hi(VOFF, 0, HW)
        dma_lo(VOFF)
        dma_hi(VOFF, HW, N)
        dma_hi(QOFF, HW, N)

        # ---- PE warm-up: junk matmuls open the p-state ramp while the
        # first input chunks land.
        wsrc = warm.tile([P, 512], bf)
        nc.vector.memset(wsrc, 0)

        def junk(n, gate=None):
            for _ in range(n):
                pw = psp.tile([P, 512], f32, tag="pp", name="junk")
                lhs = wsrc if gate is None else gate
                nc.tensor.matmul(pw, lhs[:, 0:P], wsrc, start=True, stop=True)

        # ---- helpers -----------------------------------------------------
        def qk_chunk(dst, woff, xoff, c):
            """Project one 512-token chunk of QT or KT."""
            pp = psp.tile([P, 512], f32, tag="pp", name="pp")
            nc.tensor.matmul(
                pp, wp_hi[:, woff:woff + R],
                xall_hi[:, xoff + c * 512:xoff + (c + 1) * 512],
                start=True, stop=False,
            )
            nc.tensor.matmul(
                pp, wp_lo[:, woff:woff + R],
                xall_lo[:, xoff + c * 512:xoff + (c + 1) * 512],
                start=False, stop=True,
            )
            nc.vector.tensor_copy(dst[:, ts(c, 512)], pp)

        def vl_block(j):
            """Project value block j into its fp8 pair slot [tok, 2, lat]."""
            pv = psp.tile([P, 512], f32, tag="pp", name="pv")
            nc.tensor.matmul(
                pv[:, 0:P], xall_hi[:, VOFF + j * P:VOFF + (j + 1) * P],
                wp_hi[:, LOFF:LOFF + R], start=True, stop=False,
            )
            nc.tensor.matmul(
                pv[:, 0:P], xall_lo[:, VOFF + j * P:VOFF + (j + 1) * P],
                wp_lo[:, LOFF:LOFF + R], start=False, stop=True,
            )
            nc.vector.tensor_copy(vlp[j // 2][:, j % 2, :], pv[:, 0:P])

        es = {}       # (h, pair) -> fp8 S^T tile [key 128, 2, query 1024]
        psts = {}     # (h, j) -> PSUM scores tile

        def s_mm_c(h, j, c):
            if (h, j) not in psts:
                psts[(h, j)] = psst.tile([P, HW], f32, tag="pst", name="pst")
            nc.tensor.matmul(
                psts[(h, j)][:, ts(c, 512)], kts[:, ts(j, P)],
                qts[:, h * HW + c * 512:h * HW + (c + 1) * 512],
                start=True, stop=True,
            )

        def s_mm(h, j):
            for c in range(2):
                s_mm_c(h, j, c)

        def s_exp(h, j, c=None):
            """exp of scores block j (or just its 512-col chunk c)."""
            p = j // 2
            if (h, p) not in es:
                es[(h, p)] = epool.tile([P, 2, HW], edt, tag="es",
                                        name=f"es{h}_{p}")
            if c is None:
                nc.scalar.activation(es[(h, p)][:, j % 2, :],
                                     psts.pop((h, j)), EXP)
            else:
                nc.scalar.activation(es[(h, p)][:, j % 2, ts(c, 512)],
                                     psts[(h, j)][:, ts(c, 512)], EXP)
                if c == 1:
                    psts.pop((h, j))

        pohs = {}

        def o_half(h, p, q):
            """Half of pair p's O accumulation (q=0: cols 0:512, q=1: 512:)."""
            if h not in pohs:
                pohs[h] = psoh.tile([P, HW], f32, tag="poh", name="poh")
            if USE_FP8 and USE_DR:
                # NB: on HW, start=True zeroes the accumulator's whole PSUM
                # bank, so only the first 256-chunk of each bank may set it.
                for c in (2 * q, 2 * q + 1):
                    nc.tensor.matmul(
                        pohs[h][:, ts(c, 256)],
                        vlp[p],
                        es[(h, p)][:, :, ts(c, 256)],
                        start=(p == 0 and c % 2 == 0), stop=(p == NPAIR - 1),
                        perf_mode=DR, skip_group_check=True,
                    )
            else:
                # q doubles as the within-pair block index here
                for c in range(2):
                    nc.tensor.matmul(
                        pohs[h][:, ts(c, 512)],
                        vlp[p][:, q, :],
                        es[(h, p)][:, q, ts(c, 512)],
                        start=(p == 0 and q == 0),
                        stop=(p == NPAIR - 1 and q == 1),
                        skip_group_check=True,
                    )

        ohats = {}

        def ohat_copy(h, split=False):
            oh = ohs.tile([P, HW], bf, tag=f"oh{h}", name=f"oh{h}")
            if split:
                # ACT is idle post-stream; copy halves on both engines
                nc.scalar.activation(oh[:, 0:512], pohs[h][:, 0:512], COPY)
                nc.vector.tensor_copy(oh[:, 512:HW], pohs[h][:, 512:HW])
            else:
                nc.vector.tensor_copy(oh, pohs[h])
            ohats[h] = oh

        stages = {}

        def final_ib(h, ib, mul_engine):
            """Normalize i-block ib of half h into the staging tile."""
            if h not in stages:
                stages[h] = outp.tile([P, NIB, D], bf, tag=f"st{h}",
                                      name=f"st{h}")
            po = psp.tile([P, 512], f32, tag="pp", name="po")
            nc.tensor.matmul(
                po[:, 0:EV], ohats[h][:, ts(ib, P)], rmat_s,
                start=True, stop=True,
            )
            rec = nrm.tile([P, 1], f32, tag="rec", name="rec")
            nc.vector.reciprocal(rec, po[:, D:D + 1])
            if mul_engine == "act":
                nc.scalar.activation(stages[h][:, ib, :], po[:, 0:D], COPY,
                                     scale=rec)
            elif mul_engine == "gps":
                nc.gpsimd.tensor_scalar_mul(stages[h][:, ib, :], po[:, 0:D], rec)
            else:
                nc.vector.tensor_scalar_mul(stages[h][:, ib, :], po[:, 0:D], rec)

        def out_dma(h, ib0, ib1):
            nc.sync.dma_start(
                out=out[h, :, ib0:ib1, :],
                in_=stages[h][:, ib0:ib1, :],
            )

        # ---- emission schedule ------------------------------------------
        # Pre-stream: warm the PE, then the shortest chain to the first exp:
        # kts chunk0 -> qts chunk0 -> S(0,0) col-half 0 -> exp.  The first
        # score block is processed in two 512-wide exps so the stream starts
        # as soon as the first 512 Q tokens are projected.
        junk(3)
        qk_chunk(kts, AKOFF, KOFF, 0)
        qk_chunk(qts, AQOFF, QOFF, 0)
        s_mm_c(0, 0, 0)
        s_exp(0, 0, c=0)
        qk_chunk(qts, AQOFF, QOFF, 1)
        s_mm_c(0, 0, 1)
        s_exp(0, 0, c=1)

        # h=0 stream: one exp slot per key block j; projections and the
        # value-latent blocks fill the PE slack as their DMA lands.
        for j in range(1, NB):
            s_mm(0, j)
            s_exp(0, j)
            if j == 1:
                qk_chunk(kts, AKOFF, KOFF, 1)
            elif j in (2, 3):
                junk(1)
            elif j == 4:
                qk_chunk(kts, AKOFF, KOFF, 2)
            elif j == 5:
                qk_chunk(kts, AKOFF, KOFF, 3)
            elif j in (6, 7):
                vl_block(2 * (j - 6))
                vl_block(2 * (j - 6) + 1)
            elif 8 <= j <= 12:
                vl_block(j - 4)
                o_half(0, (j - 8) // 2, (j - 8) % 2)
            elif j == 13:
                vl_block(9)
                qk_chunk(qts, AQOFF, QOFF, 2)
            elif j == 14:
                vl_block(10)
                qk_chunk(qts, AQOFF, QOFF, 3)
            elif j == 15:
                vl_block(11)
                o_half(0, 2, 1)

        # h=1 stream; remaining O(h=0), finalize h=0, and O(h=1) overlap it.
        for j in range(NB):
            s_mm(1, j)
            s_exp(1, j)
            if j <= 3:
                vl_block(12 + j)
                o_half(0, 3 + j // 2, j % 2)
            elif j <= 6:
                o_half(0, j + 1, 0)
                o_half(0, j + 1, 1)
            elif j == 7:
                ohat_copy(0)
            elif 8 <= j <= 15:
                final_ib(0, j - 8, "vec")
                o_half(1, (j - 8) // 2, (j - 8) % 2)
                if j == 12:
                    out_dma(0, 0, 4)

        # ---- tail: remaining O(h=1), bank-split: the q=0 halves complete
        # cols 0:512 of the accumulator, so their finals overlap the q=1
        # halves.  ACT is free after the exp stream.
        out_dma(0, 4, NIB)
        for p in range(4, NPAIR):
            o_half(1, p, 0)
        oh1 = ohs.tile([P, HW], bf, tag="oh1", name="oh1")
        ohats[1] = oh1
        nc.scalar.activation(oh1[:, 0:512], pohs[1][:, 0:512], COPY)
        for p in range(4, NPAIR):
            o_half(1, p, 1)
        nc.vector.tensor_copy(oh1[:, 512:HW], pohs[1][:, 512:HW])
        for ib in range(NIB):
            final_ib(1, ib, "act" if ib % 2 else "vec")
            if ib == 1:
                out_dma(1, 0, 2)
            elif ib == 3:
                out_dma(1, 2, 4)
            elif ib == 5:
                out_dma(1, 4, 6)
        out_dma(1, 6, NIB)

    return nc


def dedup_ldweights(nc):
    """Drop Ldweights instructions that reload the exact weights already in
    the PE array (same AP, nothing clobbering in between)."""
    dropped = 0
    for f in nc.m.functions:
        for blk in f.blocks:
            insts = list(blk.instructions)
            kept = []
            last_key = None
            for ins in insts:
                tname = type(ins).__name__
                if "PE" in str(getattr(ins, "engine", "")):
                    if tname == "InstLdweights":
                        ap = ins.ins[0]
                        key = (
                            ap.memref,
                            ap.offset,
                            str(ap.ap),
                            str(ap.dtype),
                            str(getattr(ins, "is_transpose", None)),
                        )
                        si = ins.sync_info
                        no_sync = si is None or (
                            len(si.on_wait) == 0 and len(si.on_update) == 0
                        )
                        if key == last_key and no_sync:
                            dropped += 1
                            continue
                        last_key = key
                    elif tname not in (
                        "InstMatmult",
                        "InstEventSemaphore",
                        "InstNoOp",
                        "InstDrain",
                    ):
                        last_key = None
                kept.append(ins)
            if len(kept) != len(insts):
                blk.instructions = kept
    return dropped


def prep_host(query, key, value, Wq, bq, Wk, bk, Wv, bv):
    """Host-side layout/algebra prep. Returns per-core input maps."""
    s = np.sqrt(np.float64(D))
    Wqp = np.concatenate([Wq, bq[:, None]], axis=1)  # [131, 132]
    Wkp = np.concatenate([Wk, bk[:, None]], axis=1)
    G = (Wqp.astype(np.float64).T @ Wkp.astype(np.float64)) / s  # [132, 132]
    U, S, Vt = np.linalg.svd(G)
    Aq = (U[:, :R] * np.sqrt(S[:R])).astype(np.float32)  # [132, 128]
    Ak = (Vt[:R, :].T * np.sqrt(S[:R])).astype(np.float32)

    W2 = np.zeros((DP, EV), np.float64)  # maps X -> [V | 1]
    W2[:D, :D] = Wv.T
    W2[D, :D] = bv
    W2[D, D] = 1.0
    U2, S2, V2t = np.linalg.svd(W2)
    L = (U2[:, :R] * np.sqrt(S2[:R])).astype(np.float32)  # [132, 128]
    Rm = (V2t[:R, :].T * np.sqrt(S2[:R])).astype(np.float32)  # [132, 128]

    wpack = np.concatenate([Aq, Ak, L], axis=1)  # [132, 384]
    wpack16 = np.ascontiguousarray(wpack.astype(_BF16))
    rmat16 = np.ascontiguousarray(Rm.T.astype(_BF16))  # [128, 132]

    ones_row = np.ones((1, N), np.float32)
    in_maps = []
    for c in range(NCORES):
        xs = [np.concatenate([x.T, ones_row], axis=0)
              for x in (query[c], key[c], value[c])]
        xallc = np.concatenate(xs, axis=1)  # [132, 6144]
        in_maps.append({
            "xall": np.ascontiguousarray(xallc.astype(_BF16)),
            "wpack": wpack16,
            "rmat": rmat16,
        })
    return in_maps


def assemble(res):
    """Depermute per-core outputs [NH, P, NIB, D] -> [NCORES, N, D] fp32."""
    outs = []
    for c in range(NCORES):
        o = np.asarray(res.results[c]["out"])  # [2, 128, 8, 131] bf16
        o = o.transpose(0, 2, 1, 3).reshape(N, D).astype(np.float32)
        outs.append(o)
    return np.stack(outs)


_NC_CACHE = {}


def _get_nc():
    if "nc" not in _NC_CACHE:
        nc = build_nc()
        if not nc.is_finalized():
            nc.finalize()
        dedup_ldweights(nc)
        _NC_CACHE["nc"] = nc
    return _NC_CACHE["nc"]


def run_on_cores(in_maps, trace=False, **kw):
    from concourse.bass_utils import run_bass_kernel_spmd

    nc = _get_nc()
    return run_bass_kernel_spmd(nc, in_maps, core_ids=list(range(NCORES)),
                                trace=trace, **kw)


def kernel(query, key, value, Wq, bq, Wk, bk, Wv, bv):
    in_maps = prep_host(query, key, value, Wq, bq, Wk, bk, Wv, bv)
    res = run_on_cores(in_maps)
    return assemble(res)
